# revision 14
# baseline (speedup 1.0000x reference)
"""Trainium2 Bass kernel for nn_Attention_79998060855419 (sparse_attention).

Pipeline per row i of node1 [131072, 512]:
    x      = concat(node1[i], u_rep)                     # [1024]
    weight = node1[i] @ lin1_w.T + lin1_b                # [1]
    alpha  = sigmoid(weight) + 1                         # in (1, 2)
    h0     = selu(x @ att1_w.T + att1_b)                 # [512]
    h1     = selu(h0 @ att2_w.T + att2_b)                # [128]
    s      = h1 @ att3_w.T + att3_b                      # [1]
    out[i] = entmax_bisect(s, alpha)  over dim of size 1 # [1]

Distribution: pure data-parallel over the neighbor axis — 8 cores x 16384
rows, MLP weights and u_rep replicated (per the sharding hint).

Device-side layout: activations flow transposed (features on partitions,
tokens on the free axis), so node1 is fed as node1.T tiles and every matmul
is weights-stationary.  Host-side prep only reshapes/transposes inputs and
folds biases/constants (u_rep contribution of layer 1, selu scale/offset
into the next layer's weights/biases) — all FLOPs over node1-derived data
run on the NeuronCores.

selu(t) = SC*relu(t) + SC*A*(exp(min(t,0)) - 1) is computed as
    e'  = exp(t + ln A)            (ScalarE, bias-folded)
    r'  = max(t, -b) + b           (VectorE tensor_scalar, = relu(t+b) pre-bias)
    nc_ = min(e', A) + r'          (VectorE scalar_tensor_tensor)
with the affine map  selu = SC*nc_ - SC*A  folded into the next layer's
weights/bias on the host.

entmax_bisect with d=1 degenerates: tau_hi == tau_lo == z - 1, dm0 == 0, so
every bisection iteration computes p = clip(z - (z-1), 0)^(1/(alpha-1)) and
the result is p / sum(p) = p / p.  The kernel computes exactly that
(z = s*(alpha-1), t = z - (z-1), p = exp(ln(t) * (1/(alpha-1))), out = p/p),
which is bit-identical to running the 50 fixed-point iterations.
"""

import math

import numpy as np

import concourse.bacc as bacc
import concourse.mybir as mybir
import concourse.tile as tile
from concourse.bass_utils import run_bass_kernel_spmd

N = 131072
D = 512
N_CORES = 8
TPC = N // N_CORES          # tokens per core = 16384
BLK = 512                   # tokens per block
NBLK = TPC // BLK           # 32 blocks per core

SC = 1.0507009873554804934193349852946   # selu scale
A = 1.6732632423543772848170429916717    # selu alpha
LN_A = math.log(A)

F32 = mybir.dt.float32
BF16 = mybir.dt.bfloat16
AF = mybir.ActivationFunctionType
ALU = mybir.AluOpType

_CACHE = {}


def _build():
    if "nc" in _CACHE:
        return _CACHE["nc"]

    nc = bacc.Bacc("TRN2", target_bir_lowering=False, debug=False,
                   num_devices=N_CORES)

    # Per-core inputs (shard of node1.T + replicated, host-folded weights).
    n1t_d = nc.dram_tensor("n1t", [D, TPC], F32, kind="ExternalInput")
    w1at_d = nc.dram_tensor("w1at", [D, D], F32, kind="ExternalInput")
    lin1t_d = nc.dram_tensor("lin1t", [D, 1], F32, kind="ExternalInput")
    w2te_d = nc.dram_tensor("w2te", [D, 128], BF16, kind="ExternalInput")
    w3te_d = nc.dram_tensor("w3te", [128, 1], BF16, kind="ExternalInput")
    be1_d = nc.dram_tensor("be1", [D, 1], F32, kind="ExternalInput")
    bnr1_d = nc.dram_tensor("bnr1", [D, 1], F32, kind="ExternalInput")
    bpr1_d = nc.dram_tensor("bpr1", [D, 1], F32, kind="ExternalInput")
    be2_d = nc.dram_tensor("be2", [128, 1], F32, kind="ExternalInput")
    bnr2_d = nc.dram_tensor("bnr2", [128, 1], F32, kind="ExternalInput")
    bpr2_d = nc.dram_tensor("bpr2", [128, 1], F32, kind="ExternalInput")
    # per-partition broadcasts of scalar consts for the entmax chain
    b3bc_d = nc.dram_tensor("b3bc", [NBLK, 1], F32, kind="ExternalInput")
    lbbc_d = nc.dram_tensor("lbbc", [NBLK, 1], F32, kind="ExternalInput")
    out_d = nc.dram_tensor("out", [TPC, 1], F32, kind="ExternalOutput")

    with tile.TileContext(nc) as tc:
        with (
            tc.tile_pool(name="wp", bufs=1) as wp,
            tc.tile_pool(name="n1p", bufs=3) as n1p,
            tc.tile_pool(name="ep", bufs=3) as ep,
            tc.tile_pool(name="rp", bufs=3) as rp,
            tc.tile_pool(name="h0p", bufs=8) as h0p,
            tc.tile_pool(name="h1p", bufs=2) as h1p,
            tc.tile_pool(name="swp", bufs=1) as swp,
            tc.tile_pool(name="stp", bufs=4) as stp,
            tc.tile_pool(name="chp", bufs=1) as chp,
            tc.tile_pool(name="ps1p", bufs=5, space="PSUM") as ps1p,
            tc.tile_pool(name="ps2p", bufs=1, space="PSUM") as ps2p,
            tc.tile_pool(name="pssp", bufs=1, space="PSUM") as pssp,
            tc.tile_pool(name="pswp", bufs=1, space="PSUM") as pswp,
        ):
            # ---- replicated weights / biases to SBUF (k-chunks along free) --
            w1a = wp.tile([128, 4 * D], F32, tag="w1a")
            nc.sync.dma_start(
                w1a[:], w1at_d[:].rearrange("(k p) m -> p k m", p=128))
            lin1 = wp.tile([128, 4], F32, tag="lin1")
            nc.sync.dma_start(
                lin1[:], lin1t_d[:].rearrange("(k p) o -> p k o", p=128))
            w2 = wp.tile([128, 4 * 128], BF16, tag="w2")
            nc.sync.dma_start(
                w2[:], w2te_d[:].rearrange("(k p) m -> p k m", p=128))
            w3 = wp.tile([128, 1], BF16, tag="w3")
            nc.sync.dma_start(w3[:], w3te_d[:])
            be1 = wp.tile([128, 4], F32, tag="be1")
            nc.sync.dma_start(
                be1[:], be1_d[:].rearrange("(k p) o -> p k o", p=128))
            bnr1 = wp.tile([128, 4], F32, tag="bnr1")
            nc.sync.dma_start(
                bnr1[:], bnr1_d[:].rearrange("(k p) o -> p k o", p=128))
            bpr1 = wp.tile([128, 4], F32, tag="bpr1")
            nc.sync.dma_start(
                bpr1[:], bpr1_d[:].rearrange("(k p) o -> p k o", p=128))
            be2 = wp.tile([128, 1], F32, tag="be2")
            nc.sync.dma_start(be2[:], be2_d[:])
            bnr2 = wp.tile([128, 1], F32, tag="bnr2")
            nc.sync.dma_start(bnr2[:], bnr2_d[:])
            bpr2 = wp.tile([128, 1], F32, tag="bpr2")
            nc.sync.dma_start(bpr2[:], bpr2_d[:])
            b3bc = wp.tile([NBLK, 1], F32, tag="b3bc")
            nc.sync.dma_start(b3bc[:], b3bc_d[:])
            lbbc = wp.tile([NBLK, 1], F32, tag="lbbc")
            nc.sync.dma_start(lbbc[:], lbbc_d[:])

            # s / w staging: row b holds tokens [b*BLK, (b+1)*BLK) of the shard
            s32 = swp.tile([NBLK, BLK], F32, tag="s32")
            w32 = swp.tile([NBLK, BLK], F32, tag="w32")

            for b in range(NBLK):
                # node1.T block: [128, (k t)] with k-chunks along free axis
                n1 = n1p.tile([128, 4 * BLK], F32, tag="n1")
                nc.sync.dma_start(
                    n1[:],
                    n1t_d[:, b * BLK:(b + 1) * BLK]
                    .rearrange("(k p) t -> p k t", p=128))

                # ---- layer 1 (+ selu)  -> h0T_nc, bf16 [feat, tokens] ------
                h0s = []
                for m in range(4):
                    ps1 = ps1p.tile([128, BLK], F32, tag="ps1")
                    for k in range(4):
                        nc.tensor.matmul(
                            ps1[:],
                            w1a[:, k * D + m * 128: k * D + (m + 1) * 128],
                            n1[:, k * BLK:(k + 1) * BLK],
                            start=(k == 0), stop=(k == 3))
                    e = ep.tile([128, BLK], BF16, tag="e")
                    nc.scalar.activation(e[:], ps1[:], AF.Exp,
                                         bias=be1[:, m:m + 1])
                    r = rp.tile([128, BLK], BF16, tag="r")
                    nc.vector.tensor_scalar(r[:], ps1[:],
                                            bnr1[:, m:m + 1], bpr1[:, m:m + 1],
                                            ALU.max, ALU.add)
                    h0 = h0p.tile([128, BLK], BF16, tag="h0")
                    nc.vector.scalar_tensor_tensor(h0[:], e[:], A, r[:],
                                                   ALU.min, ALU.add)
                    h0s.append(h0)

                # ---- lin1: w = node1 @ lin1_w.T  (M=1 matmuls) -------------
                psw = pswp.tile([1, BLK], F32, tag="psw")
                for k in range(4):
                    nc.tensor.matmul(psw[:], lin1[:, k:k + 1],
                                     n1[:, k * BLK:(k + 1) * BLK],
                                     start=(k == 0), stop=(k == 3))
                # raw w (lin1_b and negation folded into the chain's exp)
                wst = stp.tile([1, BLK], F32, tag="wst")
                nc.vector.tensor_copy(wst[:], psw[:])
                nc.sync.dma_start(w32[b:b + 1, :], wst[:])

                # ---- layer 2 (+ selu) -> h1T_nc, bf16 [128, tokens] --------
                ps2 = ps2p.tile([128, BLK], F32, tag="ps2")
                for k in range(4):
                    nc.tensor.matmul(ps2[:], w2[:, k * 128:(k + 1) * 128],
                                     h0s[k][:], start=(k == 0), stop=(k == 3))
                e2 = ep.tile([128, BLK], BF16, tag="e2")
                nc.scalar.activation(e2[:], ps2[:], AF.Exp, bias=be2[:])
                r2 = rp.tile([128, BLK], BF16, tag="r2")
                nc.vector.tensor_scalar(r2[:], ps2[:], bnr2[:], bpr2[:],
                                        ALU.max, ALU.add)
                h1 = h1p.tile([128, BLK], BF16, tag="h1")
                nc.vector.scalar_tensor_tensor(h1[:], e2[:], A, r2[:],
                                               ALU.min, ALU.add)

                # ---- layer 3: s = h1 @ att3_w.T  (M=1 matmul) --------------
                pss = pssp.tile([1, BLK], F32, tag="pss")
                nc.tensor.matmul(pss[:], w3[:], h1[:], start=True, stop=True)
                sst = stp.tile([1, BLK], F32, tag="sst")
                nc.scalar.copy(sst[:], pss[:])
                nc.sync.dma_start(s32[b:b + 1, :], sst[:])

            # ---- entmax_bisect (dim of size 1) over all tokens ------------
            # weight = w32 + lin1_b;  alpha - 1 = sigmoid(weight) = 1/d
            t1 = chp.tile([NBLK, BLK], F32, tag="t1")
            nc.scalar.activation(t1[:], w32[:], AF.Exp,
                                 bias=lbbc[:], scale=-1.0)      # e^{-weight}
            dd = chp.tile([NBLK, BLK], F32, tag="dd")
            nc.vector.tensor_scalar_add(dd[:], t1[:], 1.0)      # 1/(alpha-1)
            rd = chp.tile([NBLK, BLK], F32, tag="rd")
            nc.vector.reciprocal(rd[:], dd[:])                  # alpha-1
            z = chp.tile([NBLK, BLK], F32, tag="z")
            nc.vector.scalar_tensor_tensor(z[:], s32[:], b3bc[:], rd[:],
                                           ALU.add, ALU.mult)
            zm1 = chp.tile([NBLK, BLK], F32, tag="zm1")
            nc.vector.tensor_scalar_sub(zm1[:], z[:], 1.0)      # tau
            tq = chp.tile([NBLK, BLK], F32, tag="tq")
            nc.vector.tensor_tensor(tq[:], z[:], zm1[:], ALU.subtract)
            lq = chp.tile([NBLK, BLK], F32, tag="lq")
            nc.scalar.activation(lq[:], tq[:], AF.Ln)
            le = chp.tile([NBLK, BLK], F32, tag="le")
            nc.vector.tensor_tensor(le[:], lq[:], dd[:], ALU.mult)
            p = chp.tile([NBLK, BLK], F32, tag="p")
            nc.scalar.activation(p[:], le[:], AF.Exp)
            rp = chp.tile([NBLK, BLK], F32, tag="rp")
            nc.vector.reciprocal(rp[:], p[:])
            res = chp.tile([NBLK, BLK], F32, tag="res")
            nc.vector.tensor_tensor(res[:], p[:], rp[:], ALU.mult)

            nc.sync.dma_start(
                out_d[:].rearrange("(q t) o -> q (t o)", q=NBLK), res[:])

    nc.compile()
    _CACHE["nc"] = nc
    return nc


def _prep_host(node1, u_rep, att1_w, att1_b, att2_w, att2_b, att3_w, att3_b,
               lin1_w, lin1_b):
    import ml_dtypes
    f32 = np.float32
    node1 = np.asarray(node1, f32)
    att1_w = np.asarray(att1_w, f32)
    att2_w = np.asarray(att2_w, f32)
    att3_w = np.asarray(att3_w, f32)
    lin1_w = np.asarray(lin1_w, f32)
    u_rep = np.asarray(u_rep, f32)
    C = np.float32(SC * A)

    # layer 1: u_rep's contribution + att1_b folded into per-feature bias
    u_bias = (att1_w[:, D:] @ u_rep[0] + np.asarray(att1_b, f32)).astype(f32)
    w1at = np.ascontiguousarray(att1_w[:, :D].T)              # [D, D] f32
    be1 = (u_bias + np.float32(LN_A)).reshape(D, 1)
    bnr1 = (-u_bias).reshape(D, 1)
    bpr1 = u_bias.reshape(D, 1).copy()

    # selu affine (h = SC*nc - C) folded into layer 2
    w2te = np.ascontiguousarray(
        (SC * att2_w.T).astype(ml_dtypes.bfloat16))           # [D, 128] bf16
    b2_eff = (np.asarray(att2_b, f32) - C * att2_w.sum(axis=1)).astype(f32)
    be2 = (b2_eff + np.float32(LN_A)).reshape(128, 1)
    bnr2 = (-b2_eff).reshape(128, 1)
    bpr2 = b2_eff.reshape(128, 1).copy()

    # selu affine folded into layer 3
    w3te = np.ascontiguousarray(
        (SC * att3_w.T).astype(ml_dtypes.bfloat16))           # [128, 1] bf16
    b3_eff = np.float32(np.asarray(att3_b, f32)[0] - C * att3_w.sum())

    lin1t = np.ascontiguousarray(lin1_w.T)                    # [D, 1] f32
    b3bc = np.full((NBLK, 1), b3_eff, f32)
    lbbc = np.full((NBLK, 1), -np.float32(np.asarray(lin1_b, f32)[0]), f32)

    shared = dict(w1at=w1at, lin1t=lin1t, w2te=w2te, w3te=w3te, b3bc=b3bc,
                  lbbc=lbbc,
                  be1=np.ascontiguousarray(be1), bnr1=np.ascontiguousarray(bnr1),
                  bpr1=np.ascontiguousarray(bpr1), be2=np.ascontiguousarray(be2),
                  bnr2=np.ascontiguousarray(bnr2), bpr2=np.ascontiguousarray(bpr2))
    in_maps = []
    for c in range(N_CORES):
        m = dict(shared)
        m["n1t"] = np.ascontiguousarray(node1[c * TPC:(c + 1) * TPC, :].T)
        in_maps.append(m)
    return in_maps


def kernel(node1, u_rep, att1_w, att1_b, att2_w, att2_b, att3_w, att3_b,
           lin1_w, lin1_b, num_neighs=None, **_unused):
    nc = _build()
    in_maps = _prep_host(node1, u_rep, att1_w, att1_b, att2_w, att2_b,
                         att3_w, att3_b, lin1_w, lin1_b)
    res = run_bass_kernel_spmd(nc, in_maps, core_ids=list(range(N_CORES)))
    out = np.concatenate([res.results[c]["out"] for c in range(N_CORES)],
                         axis=0)
    return out.astype(np.float32)


# revision 20
# speedup vs baseline: 230.9264x; 230.9264x over previous
"""Trainium2 Bass kernel for nn_Attention_79998060855419 (sparse_attention).

Pipeline per row i of node1 [131072, 512]:
    x      = concat(node1[i], u_rep)                     # [1024]
    weight = node1[i] @ lin1_w.T + lin1_b                # [1]
    alpha  = sigmoid(weight) + 1                         # in (1, 2)
    h0     = selu(x @ att1_w.T + att1_b)                 # [512]
    h1     = selu(h0 @ att2_w.T + att2_b)                # [128]
    s      = h1 @ att3_w.T + att3_b                      # [1]
    out[i] = entmax_bisect(s, alpha)  over dim of size 1 # [1]

Distribution: pure data-parallel over the neighbor axis — 8 cores x 16384
rows, MLP weights and u_rep replicated (per the sharding hint).

Device-side layout: activations flow transposed (features on partitions,
tokens on the free axis), so node1 is fed as node1.T tiles and every matmul
is weights-stationary.  Host-side prep only reshapes/transposes inputs and
folds biases/constants (u_rep contribution of layer 1, selu scale/offset
into the next layer's weights/biases) — all FLOPs over node1-derived data
run on the NeuronCores.

selu(t) = SC*relu(t) + SC*A*(exp(min(t,0)) - 1) is computed as
    e'  = exp(t + ln A)            (ScalarE, bias-folded)
    r'  = max(t, -b) + b           (VectorE tensor_scalar, = relu(t+b) pre-bias)
    nc_ = min(e', A) + r'          (VectorE scalar_tensor_tensor)
with the affine map  selu = SC*nc_ - SC*A  folded into the next layer's
weights/bias on the host.

entmax_bisect with d=1 degenerates: tau_hi == tau_lo == z - 1, dm0 == 0, so
every bisection iteration computes p = clip(z - (z-1), 0)^(1/(alpha-1)) and
the result is p / sum(p) = p / p.  The kernel computes exactly that
(z = s*(alpha-1), t = z - (z-1), p = exp(ln(t) * (1/(alpha-1))), out = p/p),
which is bit-identical to running the 50 fixed-point iterations.
"""

import math

import numpy as np

import concourse.bacc as bacc
import concourse.mybir as mybir
import concourse.tile as tile
from concourse.bass_utils import run_bass_kernel_spmd

N = 131072
D = 512
N_CORES = 8
TPC = N // N_CORES          # tokens per core = 16384
BLK = 512                   # tokens per block
NBLK = TPC // BLK           # 32 blocks per core

SC = 1.0507009873554804934193349852946   # selu scale
A = 1.6732632423543772848170429916717    # selu alpha
LN_A = math.log(A)

F32 = mybir.dt.float32
F32R = mybir.dt.float32r     # fp32 single-pass PE mode (full-rate streaming)
BF16 = mybir.dt.bfloat16
AF = mybir.ActivationFunctionType
ALU = mybir.AluOpType

_CACHE = {}


def _build():
    if "nc" in _CACHE:
        return _CACHE["nc"]

    nc = bacc.Bacc("TRN2", target_bir_lowering=False, debug=False,
                   num_devices=N_CORES)

    # Per-core inputs (shard of node1.T + replicated, host-folded weights).
    n1t_d = nc.dram_tensor("n1t", [D, TPC], F32R, kind="ExternalInput")
    w1at_d = nc.dram_tensor("w1at", [D, D], F32R, kind="ExternalInput")
    lin1t_d = nc.dram_tensor("lin1t", [D, 1], F32R, kind="ExternalInput")
    w2te_d = nc.dram_tensor("w2te", [D, 128], BF16, kind="ExternalInput")
    w3te_d = nc.dram_tensor("w3te", [128, 1], BF16, kind="ExternalInput")
    be1_d = nc.dram_tensor("be1", [D, 1], F32, kind="ExternalInput")
    bnr1_d = nc.dram_tensor("bnr1", [D, 1], F32, kind="ExternalInput")
    bpr1_d = nc.dram_tensor("bpr1", [D, 1], F32, kind="ExternalInput")
    be2_d = nc.dram_tensor("be2", [128, 1], F32, kind="ExternalInput")
    bnr2_d = nc.dram_tensor("bnr2", [128, 1], F32, kind="ExternalInput")
    bpr2_d = nc.dram_tensor("bpr2", [128, 1], F32, kind="ExternalInput")
    # per-partition broadcasts of scalar consts for the entmax chain
    b3bc_d = nc.dram_tensor("b3bc", [NBLK, 1], F32, kind="ExternalInput")
    lbbc_d = nc.dram_tensor("lbbc", [NBLK, 1], F32, kind="ExternalInput")
    out_d = nc.dram_tensor("out", [TPC, 1], F32, kind="ExternalOutput")

    with tile.TileContext(nc) as tc:
        with (
            tc.tile_pool(name="wp", bufs=1) as wp,
            tc.tile_pool(name="n1p", bufs=3) as n1p,
            tc.tile_pool(name="ep", bufs=3) as ep,
            tc.tile_pool(name="rp", bufs=3) as rp,
            tc.tile_pool(name="h0p", bufs=8) as h0p,
            tc.tile_pool(name="h1p", bufs=2) as h1p,
            tc.tile_pool(name="swp", bufs=1) as swp,
            tc.tile_pool(name="stp", bufs=4) as stp,
            tc.tile_pool(name="chp", bufs=1) as chp,
            tc.tile_pool(name="ps1p", bufs=5, space="PSUM") as ps1p,
            tc.tile_pool(name="ps2p", bufs=1, space="PSUM") as ps2p,
            tc.tile_pool(name="pssp", bufs=1, space="PSUM") as pssp,
            tc.tile_pool(name="pswp", bufs=1, space="PSUM") as pswp,
        ):
            # ---- replicated weights / biases to SBUF (k-chunks along free) --
            w1a = wp.tile([128, 4 * D], F32R, tag="w1a")
            nc.sync.dma_start(
                w1a[:], w1at_d[:].rearrange("(k p) m -> p k m", p=128))
            lin1 = wp.tile([128, 4], F32R, tag="lin1")
            nc.sync.dma_start(
                lin1[:], lin1t_d[:].rearrange("(k p) o -> p k o", p=128))
            w2 = wp.tile([128, 4 * 128], BF16, tag="w2")
            nc.sync.dma_start(
                w2[:], w2te_d[:].rearrange("(k p) m -> p k m", p=128))
            w3 = wp.tile([128, 1], BF16, tag="w3")
            nc.sync.dma_start(w3[:], w3te_d[:])
            be1 = wp.tile([128, 4], F32, tag="be1")
            nc.sync.dma_start(
                be1[:], be1_d[:].rearrange("(k p) o -> p k o", p=128))
            bnr1 = wp.tile([128, 4], F32, tag="bnr1")
            nc.sync.dma_start(
                bnr1[:], bnr1_d[:].rearrange("(k p) o -> p k o", p=128))
            bpr1 = wp.tile([128, 4], F32, tag="bpr1")
            nc.sync.dma_start(
                bpr1[:], bpr1_d[:].rearrange("(k p) o -> p k o", p=128))
            be2 = wp.tile([128, 1], F32, tag="be2")
            nc.sync.dma_start(be2[:], be2_d[:])
            bnr2 = wp.tile([128, 1], F32, tag="bnr2")
            nc.sync.dma_start(bnr2[:], bnr2_d[:])
            bpr2 = wp.tile([128, 1], F32, tag="bpr2")
            nc.sync.dma_start(bpr2[:], bpr2_d[:])
            b3bc = wp.tile([NBLK, 1], F32, tag="b3bc")
            nc.sync.dma_start(b3bc[:], b3bc_d[:])
            lbbc = wp.tile([NBLK, 1], F32, tag="lbbc")
            nc.sync.dma_start(lbbc[:], lbbc_d[:])

            # s / w staging: row b holds tokens [b*BLK, (b+1)*BLK) of the shard
            s32 = swp.tile([NBLK, BLK], F32, tag="s32")
            w32 = swp.tile([NBLK, BLK], F32, tag="w32")

            for b in range(NBLK):
                # node1.T block: [128, (k t)] with k-chunks along free axis
                n1 = n1p.tile([128, 4 * BLK], F32R, tag="n1")
                nc.sync.dma_start(
                    n1[:],
                    n1t_d[:, b * BLK:(b + 1) * BLK]
                    .rearrange("(k p) t -> p k t", p=128))

                # ---- layer 1 (+ selu)  -> h0T_nc, bf16 [feat, tokens] ------
                h0s = []
                for m in range(4):
                    ps1 = ps1p.tile([128, BLK], F32, tag="ps1")
                    for k in range(4):
                        nc.tensor.matmul(
                            ps1[:],
                            w1a[:, k * D + m * 128: k * D + (m + 1) * 128],
                            n1[:, k * BLK:(k + 1) * BLK],
                            start=(k == 0), stop=(k == 3))
                    e = ep.tile([128, BLK], BF16, tag="e")
                    nc.scalar.activation(e[:], ps1[:], AF.Exp,
                                         bias=be1[:, m:m + 1])
                    q = rp.tile([128, BLK], BF16, tag="q")
                    nc.vector.tensor_scalar(q[:], e[:],
                                            A, bpr1[:, m:m + 1],
                                            ALU.min, ALU.add)
                    h0 = h0p.tile([128, BLK], BF16, tag="h0")
                    nc.vector.scalar_tensor_tensor(h0[:], ps1[:],
                                                   bnr1[:, m:m + 1], q[:],
                                                   ALU.max, ALU.add)
                    h0s.append(h0)

                # ---- lin1: w = node1 @ lin1_w.T  (M=1 matmuls) -------------
                psw = pswp.tile([1, BLK], F32, tag="psw")
                for k in range(4):
                    nc.tensor.matmul(psw[:], lin1[:, k:k + 1],
                                     n1[:, k * BLK:(k + 1) * BLK],
                                     start=(k == 0), stop=(k == 3))
                # raw w (lin1_b and negation folded into the chain's exp)
                wst = stp.tile([1, BLK], F32, tag="wst")
                nc.scalar.copy(wst[:], psw[:])
                nc.sync.dma_start(w32[b:b + 1, :], wst[:])

                # ---- layer 2 (+ selu) -> h1T_nc, bf16 [128, tokens] --------
                ps2 = ps2p.tile([128, BLK], F32, tag="ps2")
                for k in range(4):
                    nc.tensor.matmul(ps2[:], w2[:, k * 128:(k + 1) * 128],
                                     h0s[k][:], start=(k == 0), stop=(k == 3))
                e2 = ep.tile([128, BLK], BF16, tag="e2")
                nc.scalar.activation(e2[:], ps2[:], AF.Exp, bias=be2[:])
                q2 = rp.tile([128, BLK], BF16, tag="q2")
                nc.vector.tensor_scalar(q2[:], e2[:], A, bpr2[:],
                                        ALU.min, ALU.add)
                h1 = h1p.tile([128, BLK], BF16, tag="h1")
                nc.vector.scalar_tensor_tensor(h1[:], ps2[:], bnr2[:], q2[:],
                                               ALU.max, ALU.add)

                # ---- layer 3: s = h1 @ att3_w.T  (M=1 matmul) --------------
                pss = pssp.tile([1, BLK], F32, tag="pss")
                nc.tensor.matmul(pss[:], w3[:], h1[:], start=True, stop=True)
                sst = stp.tile([1, BLK], F32, tag="sst")
                nc.scalar.copy(sst[:], pss[:])
                nc.sync.dma_start(s32[b:b + 1, :], sst[:])

            # ---- entmax_bisect (dim of size 1) over all tokens ------------
            # weight = w32 + lin1_b;  alpha - 1 = sigmoid(weight) = 1/d
            t1 = chp.tile([NBLK, BLK], F32, tag="t1")
            nc.scalar.activation(t1[:], w32[:], AF.Exp,
                                 bias=lbbc[:], scale=-1.0)      # e^{-weight}
            dd = chp.tile([NBLK, BLK], F32, tag="dd")
            nc.vector.tensor_scalar_add(dd[:], t1[:], 1.0)      # 1/(alpha-1)
            rd = chp.tile([NBLK, BLK], F32, tag="rd")
            nc.vector.reciprocal(rd[:], dd[:])                  # alpha-1
            z = chp.tile([NBLK, BLK], F32, tag="z")
            nc.vector.scalar_tensor_tensor(z[:], s32[:], b3bc[:], rd[:],
                                           ALU.add, ALU.mult)
            zm1 = chp.tile([NBLK, BLK], F32, tag="zm1")
            nc.vector.tensor_scalar_sub(zm1[:], z[:], 1.0)      # tau
            tq = chp.tile([NBLK, BLK], F32, tag="tq")
            nc.vector.tensor_tensor(tq[:], z[:], zm1[:], ALU.subtract)
            lq = chp.tile([NBLK, BLK], F32, tag="lq")
            nc.scalar.activation(lq[:], tq[:], AF.Ln)
            le = chp.tile([NBLK, BLK], F32, tag="le")
            nc.vector.tensor_tensor(le[:], lq[:], dd[:], ALU.mult)
            p = chp.tile([NBLK, BLK], F32, tag="p")
            nc.scalar.activation(p[:], le[:], AF.Exp)
            rp = chp.tile([NBLK, BLK], F32, tag="rp")
            nc.vector.reciprocal(rp[:], p[:])
            res = chp.tile([NBLK, BLK], F32, tag="res")
            nc.vector.tensor_tensor(res[:], p[:], rp[:], ALU.mult)

            nc.sync.dma_start(
                out_d[:].rearrange("(q t) o -> q (t o)", q=NBLK), res[:])

    nc.compile()
    _CACHE["nc"] = nc
    return nc


def _prep_host(node1, u_rep, att1_w, att1_b, att2_w, att2_b, att3_w, att3_b,
               lin1_w, lin1_b):
    import ml_dtypes
    f32 = np.float32
    node1 = np.asarray(node1, f32)
    att1_w = np.asarray(att1_w, f32)
    att2_w = np.asarray(att2_w, f32)
    att3_w = np.asarray(att3_w, f32)
    lin1_w = np.asarray(lin1_w, f32)
    u_rep = np.asarray(u_rep, f32)
    C = np.float32(SC * A)

    # layer 1: u_rep's contribution + att1_b folded into per-feature bias
    u_bias = (att1_w[:, D:] @ u_rep[0] + np.asarray(att1_b, f32)).astype(f32)
    w1at = np.ascontiguousarray(att1_w[:, :D].T)              # [D, D] f32
    be1 = (u_bias + np.float32(LN_A)).reshape(D, 1)
    bnr1 = (-u_bias).reshape(D, 1)
    bpr1 = u_bias.reshape(D, 1).copy()

    # selu affine (h = SC*nc - C) folded into layer 2
    w2te = np.ascontiguousarray(
        (SC * att2_w.T).astype(ml_dtypes.bfloat16))           # [D, 128] bf16
    b2_eff = (np.asarray(att2_b, f32) - C * att2_w.sum(axis=1)).astype(f32)
    be2 = (b2_eff + np.float32(LN_A)).reshape(128, 1)
    bnr2 = (-b2_eff).reshape(128, 1)
    bpr2 = b2_eff.reshape(128, 1).copy()

    # selu affine folded into layer 3
    w3te = np.ascontiguousarray(
        (SC * att3_w.T).astype(ml_dtypes.bfloat16))           # [128, 1] bf16
    b3_eff = np.float32(np.asarray(att3_b, f32)[0] - C * att3_w.sum())

    lin1t = np.ascontiguousarray(lin1_w.T)                    # [D, 1] f32
    b3bc = np.full((NBLK, 1), b3_eff, f32)
    lbbc = np.full((NBLK, 1), -np.float32(np.asarray(lin1_b, f32)[0]), f32)

    shared = dict(w1at=w1at, lin1t=lin1t, w2te=w2te, w3te=w3te, b3bc=b3bc,
                  lbbc=lbbc,
                  be1=np.ascontiguousarray(be1), bnr1=np.ascontiguousarray(bnr1),
                  bpr1=np.ascontiguousarray(bpr1), be2=np.ascontiguousarray(be2),
                  bnr2=np.ascontiguousarray(bnr2), bpr2=np.ascontiguousarray(bpr2))
    in_maps = []
    for c in range(N_CORES):
        m = dict(shared)
        m["n1t"] = np.ascontiguousarray(node1[c * TPC:(c + 1) * TPC, :].T)
        in_maps.append(m)
    return in_maps


def kernel(node1, u_rep, att1_w, att1_b, att2_w, att2_b, att3_w, att3_b,
           lin1_w, lin1_b, num_neighs=None, **_unused):
    nc = _build()
    in_maps = _prep_host(node1, u_rep, att1_w, att1_b, att2_w, att2_b,
                         att3_w, att3_b, lin1_w, lin1_b)
    res = run_bass_kernel_spmd(nc, in_maps, core_ids=list(range(N_CORES)))
    out = np.concatenate([res.results[c]["out"] for c in range(N_CORES)],
                         axis=0)
    return out.astype(np.float32)
